# revision 6
# baseline (speedup 1.0000x reference)
"""Bahdanau attention on 8 TRN2 NeuronCores.

Sharding: data-parallel over batch B=32 -> 4 batches per core. Each core holds
the full (small) projection weights and computes its batch shard's projection,
softmax and context locally. No collectives.

Per-core program (slot s = one of 4 batches, t processed in 512-blocks of four
128-chunks, only up to that slot's valid length):
  1. SWDGE DMA loads enc[s, block] HBM f32 -> SBUF f32r (rounds to FP32R
     in-flight; FP32R streams through the PE at 1 cyc/row vs 4 for f32,
     with ~13-bit mantissa).
  2. PE transposes each [128t x 128e] tile -> encT (e on partitions).
  3. proj^T[h, t] = sum_e W_enc[e, h] * encT[e, t]   (W stationary, PSUM acc)
  4. ACT: x^T = tanh(proj^T + dec[h, s] + b_enc[h])  (bias is per-partition)
  5. PE: erg^T[t-chunk, 1] = x^T_chunk.T @ w_w       (plain fp32, N=1)
  After all blocks of the slot:
  6. softmax over t without max-subtraction (|erg| <= ~15 so exp is safe):
     additive -1e30 mask, exp with fused free-dim accumulation, ones-matmul
     partition reduction, reciprocal, ones-matmul broadcast, scale.
  7. PE: ctx[1, e] += alphaT_chunk.T @ enc_nat_chunk  (alpha stationary)
  8. alphaT transposed back via PE for a contiguous DMA store.

Batches are sorted by valid length on the host and dealt round-robin so each
slot's 8 batches (one per core) have similar lengths; the program is
specialized (and cached) per 4-tuple of slot chunk counts.
"""

import sys

sys.path.insert(0, "/opt/trn_rl_repo")

import math
from contextlib import ExitStack

import numpy as np

B, T, E, D, H = 32, 2048, 512, 1024, 128
NCORES = 8
SLOTS = B // NCORES  # 4
P = 128
TCH = T // P  # 16 chunks per batch
NEG = -1.0e30

_PROGRAM_CACHE = {}

# set by test.py for profiling
PROFILE = False
LAST_EXEC_NS = None
LAST_RESULTS = None


def _build_program(nbs, stage=99):
    """nbs: tuple of 4 ints, chunk count (1..16) processed per slot."""
    import concourse.mybir as mybir
    import concourse.tile as tile
    from concourse import bacc
    from concourse.masks import make_identity

    F32 = mybir.dt.float32
    F32R = mybir.dt.float32r
    AF = mybir.ActivationFunctionType

    nc = bacc.Bacc(None, target_bir_lowering=False)

    enc = nc.dram_tensor("enc", [SLOTS, T, E], F32, kind="ExternalInput")
    decT = nc.dram_tensor("decT", [D, SLOTS], F32, kind="ExternalInput")
    wenc = nc.dram_tensor("wenc", [E, H], F32, kind="ExternalInput")
    wdec = nc.dram_tensor("wdec", [D, H], F32, kind="ExternalInput")
    ww = nc.dram_tensor("ww", [H, 1], F32, kind="ExternalInput")
    benc = nc.dram_tensor("benc", [H, 1], F32, kind="ExternalInput")
    # additive mask, host-packed transposed: [p, s*16+k] = 0 or NEG
    maskT = nc.dram_tensor("maskT", [P, SLOTS * TCH], F32, kind="ExternalInput")
    ctx_out = nc.dram_tensor("ctx", [SLOTS, E], F32, kind="ExternalOutput")
    alpha_out = nc.dram_tensor("alpha", [SLOTS, T], F32, kind="ExternalOutput")

    with tile.TileContext(nc) as tc, ExitStack() as ctx:
        consts = ctx.enter_context(tc.tile_pool(name="consts", bufs=1))
        enc_pool = ctx.enter_context(tc.tile_pool(name="encp", bufs=6))
        encT_pool = ctx.enter_context(tc.tile_pool(name="encTp", bufs=8))
        xt_pool = ctx.enter_context(tc.tile_pool(name="xtp", bufs=3))
        small = ctx.enter_context(tc.tile_pool(name="small", bufs=4))
        outp = ctx.enter_context(tc.tile_pool(name="outp", bufs=1))
        ps_encT = ctx.enter_context(tc.tile_pool(name="ps_encT", bufs=2, space="PSUM"))
        ps_proj = ctx.enter_context(tc.tile_pool(name="ps_proj", bufs=2, space="PSUM"))
        ps_erg = ctx.enter_context(tc.tile_pool(name="ps_erg", bufs=2, space="PSUM"))
        ps_ctx = ctx.enter_context(tc.tile_pool(name="ps_ctx", bufs=1, space="PSUM"))
        ps_small = ctx.enter_context(tc.tile_pool(name="ps_sm", bufs=1, space="PSUM"))

        # ---- constants ----
        wenc_sb = consts.tile([P, E // P, H], F32R)
        nc.gpsimd.dma_start(wenc_sb, wenc.rearrange("(q p) h -> p q h", p=P))
        wdec_sb = consts.tile([P, D // P, H], F32)
        nc.sync.dma_start(wdec_sb, wdec.rearrange("(c p) h -> p c h", p=P))
        decT_sb = consts.tile([P, D // P, SLOTS], F32)
        nc.sync.dma_start(decT_sb, decT.rearrange("(c p) b -> p c b", p=P))
        ww_sb = consts.tile([P, 1], F32)
        nc.sync.dma_start(ww_sb, ww[:])
        benc_sb = consts.tile([P, 1], F32)
        nc.sync.dma_start(benc_sb, benc[:])
        maskT_sb = consts.tile([P, SLOTS, TCH], F32)
        nc.sync.dma_start(maskT_sb, maskT.rearrange("p (s k) -> p s k", s=SLOTS))
        ident_f32 = consts.tile([P, P], F32)
        make_identity(nc, ident_f32)
        ident = consts.tile([P, P], F32R)
        nc.vector.tensor_copy(ident, ident_f32)
        ones_col = consts.tile([P, 1], F32)
        nc.vector.memset(ones_col, 1.0)
        ones_row = consts.tile([1, P], F32)
        nc.vector.memset(ones_row, 1.0)

        # ---- decoder projection: bias_sb[h, s] = dec[s] @ W_dec + b_enc ----
        dec_ps = ps_small.tile([P, SLOTS], F32, tag="sm")
        for c in range(D // P):
            nc.tensor.matmul(
                dec_ps,
                wdec_sb[:, c, :],
                decT_sb[:, c, :],
                start=(c == 0),
                stop=(c == D // P - 1),
            )
        bias_sb = consts.tile([P, SLOTS], F32)
        nc.vector.tensor_scalar_add(bias_sb, dec_ps, benc_sb)

        # ---- output gather tiles ----
        if stage >= 8:
            ctx_all = outp.tile([1, SLOTS * E], F32)
        if stage >= 9:
            alpha_all = outp.tile([TCH, SLOTS, P], F32)

        for s in range(SLOTS):
            nch = nbs[s]
            nblocks = math.ceil(nch / 4)
            erg_ps = ps_erg.tile([P, TCH], F32, tag="erg")
            enc_tiles = []
            if stage < 2:
                continue
            for j in range(nblocks):
                c0 = 4 * j
                w = min(4, nch - c0)
                et = enc_pool.tile([P, 4, E], F32R, tag="enc")
                nc.gpsimd.dma_start(
                    et[:, :w, :],
                    enc[s, c0 * P : (c0 + w) * P, :].rearrange(
                        "(c p) e -> p c e", p=P
                    ),
                )
                enc_tiles.append((et, w))
                if stage < 3:
                    continue
                # transpose the block: encT[q][e, t] tiles
                eTs = []
                for q in range(E // P):
                    pt = ps_encT.tile([P, 4 * P], F32R, tag="encT_ps")
                    for ci in range(w):
                        nc.tensor.transpose(
                            pt[:, ci * P : (ci + 1) * P],
                            et[:, ci, q * P : (q + 1) * P],
                            ident,
                        )
                    eT = encT_pool.tile([P, 4 * P], F32R, tag="encT")
                    if q % 2 == 0:
                        nc.vector.tensor_copy(eT[:, : w * P], pt[:, : w * P])
                    else:
                        nc.scalar.copy(eT[:, : w * P], pt[:, : w * P])
                    eTs.append(eT)
                if stage < 4:
                    continue
                # projection (W stationary, accumulate over e-chunks)
                pj = ps_proj.tile([P, 4 * P], F32, tag="proj")
                for q in range(E // P):
                    nc.tensor.matmul(
                        pj[:, : w * P],
                        wenc_sb[:, q, :],
                        eTs[q][:, : w * P],
                        start=(q == 0),
                        stop=(q == E // P - 1),
                    )
                if stage < 5:
                    continue
                # tanh(proj + dec + b_enc)
                xt = xt_pool.tile([P, 4 * P], F32, tag="xt")
                nc.scalar.activation(
                    xt[:, : w * P],
                    pj[:, : w * P],
                    AF.Tanh,
                    bias=bias_sb[:, s : s + 1],
                    scale=1.0,
                )
                if stage < 6:
                    continue
                # erg^T columns
                for ci in range(w):
                    nc.tensor.matmul(
                        erg_ps[:, c0 + ci : c0 + ci + 1],
                        xt[:, ci * P : (ci + 1) * P],
                        ww_sb,
                        start=True,
                        stop=True,
                    )

            if stage < 7:
                continue
            # ---- softmax (no max-subtraction; additive mask) ----
            ergm = small.tile([P, TCH], F32, tag="ergm")
            nc.vector.tensor_add(
                ergm[:, :nch], erg_ps[:, :nch], maskT_sb[:, s, :nch]
            )
            expt = small.tile([P, TCH], F32, tag="expt")
            rowsum = small.tile([P, 1], F32, tag="rowsum")
            nc.scalar.activation(
                expt[:, :nch], ergm[:, :nch], AF.Exp, accum_out=rowsum
            )
            tot_ps = ps_small.tile([1, 1], F32, tag="sm")
            nc.tensor.matmul(tot_ps, rowsum, ones_col, start=True, stop=True)
            tot_sb = small.tile([1, 1], F32, tag="tot_sb")
            nc.vector.tensor_copy(tot_sb, tot_ps)
            rec = small.tile([1, 1], F32, tag="rec")
            nc.vector.reciprocal(rec, tot_sb)
            bc_ps = ps_small.tile([P, 1], F32, tag="sm")
            nc.tensor.matmul(bc_ps, ones_row, rec, start=True, stop=True)
            bc_sb = small.tile([P, 1], F32, tag="bc_sb")
            nc.vector.tensor_copy(bc_sb, bc_ps)
            alphaT = small.tile([P, TCH], F32R, tag="alphaT")
            nc.vector.tensor_scalar_mul(alphaT[:, :nch], expt[:, :nch], bc_sb)

            if stage < 8:
                continue
            # ---- context ----
            cx_ps = ps_ctx.tile([1, E], F32, tag="cx")
            k = 0
            for et, w in enc_tiles:
                for ci in range(w):
                    nc.tensor.matmul(
                        cx_ps,
                        alphaT[:, k : k + 1],
                        et[:, ci, :],
                        start=(k == 0),
                        stop=(k == nch - 1),
                    )
                    k += 1
            nc.vector.tensor_copy(ctx_all[:, s * E : (s + 1) * E], cx_ps)

            if stage < 9:
                continue
            # ---- alpha back to natural layout + store ----
            an_ps = ps_small.tile([TCH, P], F32R, tag="sm")
            nc.tensor.transpose(an_ps[:nch, :], alphaT[:, :nch], ident)
            nc.scalar.copy(alpha_all[:nch, s, :], an_ps[:nch, :])
            nc.sync.dma_start(
                alpha_out[s].rearrange("(k p) -> k p", p=P)[:nch],
                alpha_all[:nch, s, :],
            )

        if stage >= 8:
            nc.sync.dma_start(ctx_out.rearrange("b e -> (b e)")[None, :], ctx_all[0:1, :])

    nc.finalize()
    return nc


def _get_program(nbs):
    nbs = tuple(int(x) for x in nbs)
    if nbs not in _PROGRAM_CACHE:
        _PROGRAM_CACHE[nbs] = _build_program(nbs)
    return _PROGRAM_CACHE[nbs]


def kernel(encoder_outs, src_lens, decoder_state, mask, W_enc, b_enc, W_dec, w_w, w_b):
    global LAST_EXEC_NS, LAST_RESULTS
    from concourse.bass_utils import run_bass_kernel_spmd

    encoder_outs = np.ascontiguousarray(np.asarray(encoder_outs, dtype=np.float32))
    decoder_state = np.asarray(decoder_state, dtype=np.float32)
    mask_np = np.asarray(mask).astype(bool)
    W_enc_np = np.ascontiguousarray(np.asarray(W_enc, dtype=np.float32))
    b_enc_np = np.asarray(b_enc, dtype=np.float32).reshape(H, 1)
    W_dec_np = np.ascontiguousarray(np.asarray(W_dec, dtype=np.float32))
    w_w_np = np.ascontiguousarray(np.asarray(w_w, dtype=np.float32).reshape(H, 1))
    w_b_np = np.asarray(w_b, dtype=np.float32)  # cancels in softmax; unused

    # valid length per batch from the mask (mask True = padded)
    any_masked = mask_np.any(axis=1)
    first_masked = mask_np.argmax(axis=1)
    vlen = np.where(any_masked, first_masked, T).astype(np.int64)

    # sort batches by length desc; slot s holds ranks [8s, 8s+8)
    order = np.argsort(-vlen, kind="stable")
    nbs = []
    for s in range(SLOTS):
        grp = order[s * NCORES : (s + 1) * NCORES]
        nbs.append(max(1, int(math.ceil(vlen[grp].max() / P))))
    nbs = tuple(nbs)

    nc = _get_program(nbs)

    # additive mask in transposed layout, per assigned batch
    in_maps = []
    batch_ids = np.empty((NCORES, SLOTS), dtype=np.int64)
    for c in range(NCORES):
        ids = [int(order[s * NCORES + c]) for s in range(SLOTS)]
        batch_ids[c] = ids
        maskT = np.zeros((P, SLOTS * TCH), dtype=np.float32)
        for s, b in enumerate(ids):
            m = np.where(mask_np[b], np.float32(NEG), np.float32(0.0))
            maskT[:, s * TCH : (s + 1) * TCH] = m.reshape(TCH, P).T
        in_maps.append(
            {
                "enc": encoder_outs[ids],
                "decT": np.ascontiguousarray(decoder_state[ids].T),
                "wenc": W_enc_np,
                "wdec": W_dec_np,
                "ww": w_w_np,
                "benc": b_enc_np,
                "maskT": maskT,
            }
        )

    res = run_bass_kernel_spmd(
        nc, in_maps, list(range(NCORES)), trace=PROFILE
    )
    LAST_EXEC_NS = res.exec_time_ns
    LAST_RESULTS = res

    ctx_full = np.zeros((B, E), dtype=np.float32)
    alpha_full = np.zeros((B, T), dtype=np.float32)
    for c in range(NCORES):
        r = res.results[c]
        for s in range(SLOTS):
            b = batch_ids[c, s]
            ctx_full[b] = r["ctx"][s]
            alpha_full[b] = r["alpha"][s]
    return ctx_full, alpha_full
